# revision 12
# baseline (speedup 1.0000x reference)
"""Trainium2 Bass kernel for nn_DarcyResidual (P=256, B=128, 8 NeuronCores).

Math (reference):
    a = (x0 + 1.5) / 0.2,  p = (x1 + 0.9) / 115
    residual = -a*(p_d00 + p_d11) - a_d0*p_d0 - a_d1*p_d1 - 1
2nd-order central differences inside, 2nd-order one-sided at borders,
h = 1/256 on both axes.

Folded form computed here (G = 5/(460 h^2)):
    residual = -G * [ (X0 + 1.5)*U4 + S1*R1 + C1a*C1p ] - 1
      U4  = 4*(rowD2raw(X1) + colD2raw(X1))   (raw h^2-scaled 2nd diffs)
      R1  = rowD1raw(X1), S1 = rowD1raw(X0)   (raw 2h-scaled 1st diffs)
      C1p = colD1raw(X1), C1a = colD1raw(X0)

v2 layout per core (16 images): SBUF [partition = row-within-128-block,
free = (row-block k:2, image b, col j:256)], 8 chunks of 2 images.
All row-direction (d0) stencils are bf16 TensorE matmuls (banded stencil
matrices as lhsT blocks); the column Laplacian rides the same PSUM
accumulation via 4I matmuls on +-1-column-shifted rhs views of a padded
bf16 x1 tile.  S1*R1 is a DVE stt directly from PSUM (shifted bf16 out);
C1a*C1p comes from shifted-aligned 2x bf16 DVE stencils.  The three
terms are summed in a PSUM "res" bank by two identity bf16 matmuls
(rhs = tm and the shifted gradient sum), and ScalarE does the single
fused evacuate+affine(-G,-1)+bf16-cast.  Border columns j=0,255 keep
the f32r edge pipeline fed by a host-pregathered edge tensor.  Output
is bf16 (upcast on host); output DMAs ride the GPSIMD (SWDGE) queue.
"""

import numpy as np

P = 256
B = 128
NCORES = 8
BPC = B // NCORES          # images per core = 16
CHUNKS = 8
BCH = BPC // CHUNKS        # images per chunk = 2
FCH = 2 * BCH * P          # chunk free size = 1024
GAMMA = 5.0 * 65536.0 / 460.0

_cache = {}


def _stencils():
    D1 = np.zeros((P, P), dtype=np.float64)
    for i in range(1, P - 1):
        D1[i, i - 1] = -1.0
        D1[i, i + 1] = 1.0
    D1[0, 0:3] = [-3.0, 4.0, -1.0]
    D1[P - 1, P - 3:P] = [1.0, -4.0, 3.0]

    D2 = np.zeros((P, P), dtype=np.float64)
    for i in range(1, P - 1):
        D2[i, i - 1] = 1.0
        D2[i, i] = -2.0
        D2[i, i + 1] = 1.0
    D2[0, 0:4] = [2.0, -5.0, 4.0, -1.0]
    D2[P - 1, P - 4:P] = [-1.0, 4.0, -5.0, 2.0]
    return D1, D2


def _weights_bf16():
    """[128, 14, 128] bf16 lhsT blocks for all matmuls.
    0-3: D1 blocks; 4-7: 4*(D2-2I) blocks; 8: 4I; 9: I;
    10-13: 4*D2 blocks (edge pipeline, col stencil complete).
    All entries are small integers -> exact in bf16."""
    import ml_dtypes
    D1, D2 = _stencils()
    WR2 = 4.0 * (D2 - 2.0 * np.eye(P))
    WR2E = 4.0 * D2
    wtb = np.zeros((128, 14, 128), dtype=np.float64)
    for m in range(2):
        for kb in range(2):
            blk = lambda W: W[m * 128:(m + 1) * 128, kb * 128:(kb + 1) * 128].T
            wtb[:, m * 2 + kb, :] = blk(D1)
            wtb[:, 4 + m * 2 + kb, :] = blk(WR2)
            wtb[:, 10 + m * 2 + kb, :] = blk(WR2E)
    wtb[:, 8, :] = 4.0 * np.eye(128)
    wtb[:, 9, :] = np.eye(128)
    return wtb.astype(ml_dtypes.bfloat16)


def _build_program():
    from concourse import bacc
    import concourse.mybir as mybir
    from concourse.tile import TileContext

    f32 = mybir.dt.float32
    f32r = mybir.dt.float32r
    bf16 = mybir.dt.bfloat16
    ADD = mybir.AluOpType.add
    SUB = mybir.AluOpType.subtract
    MUL = mybir.AluOpType.mult
    COPY = mybir.ActivationFunctionType.Copy

    nc = bacc.Bacc("TRN2", target_bir_lowering=False, debug=False,
                   num_devices=NCORES)
    xe = nc.dram_tensor("xe", [128, 2, 2, BPC, 8], bf16, kind="ExternalInput")
    xb = nc.dram_tensor("xb", [128, 2, 2, BPC, P], bf16, kind="ExternalInput")
    wtbd = nc.dram_tensor("wtbd", [128, 14, 128], bf16, kind="ExternalInput")
    yout = nc.dram_tensor("yout", [128, 2, BPC, P], bf16, kind="ExternalOutput")

    with TileContext(nc) as tc:
        with (
            tc.tile_pool(name="const", bufs=1) as cpool,
            tc.tile_pool(name="edge", bufs=1) as epool,
            tc.tile_pool(name="work", bufs=2) as pool,
            tc.tile_pool(name="psum", bufs=2, space="PSUM") as pp,
        ):
            # chunk-0 bf16 inputs first, then the small weight tensors.
            # x1 goes into a 2-left-padded tile so the +-1-column-shifted
            # identity-matmul rhs views exist and the C1p stencil views
            # stay 4-byte aligned (2x mode).
            Xp0 = pool.tile([128, FCH + 4], bf16, tag="x1", bufs=3)
            nc.sync.dma_start(
                out=Xp0[:, 2:FCH + 2].rearrange(
                    "p (k b j) -> p k b j", k=2, b=BCH),
                in_=xb[:, 1, :, 0:BCH, :])
            X0c0 = pool.tile([128, 2, BCH, P], bf16, tag="x0", bufs=3)
            nc.sync.dma_start(out=X0c0[:], in_=xb[:, 0, :, 0:BCH, :])
            wtb = cpool.tile([128, 14, 128], bf16)
            nc.sync.dma_start(out=wtb[:], in_=wtbd[:])

            def Wb(i):
                return wtb[:, i, :]

            stt = nc.vector.scalar_tensor_tensor

            # ------------- edge pipeline (output cols j=0 and j=255) -------
            X0e = epool.tile([128, 2, BPC, 8], bf16)
            X1e = epool.tile([128, 2, BPC, 8], bf16)
            nc.sync.dma_start(out=X0e[:], in_=xe[:, 0])
            nc.sync.dma_start(out=X1e[:], in_=xe[:, 1])

            X0ef = X0e.rearrange("p k b c -> p (k b c)")
            X1ef = X1e.rearrange("p k b c -> p (k b c)")
            # [128, 32, 8] views
            E1 = X1e.rearrange("p k b c -> p (k b) c")
            E0 = X0e.rearrange("p k b c -> p (k b) c")

            def et(name, d=2):
                return epool.tile([128, 2 * BPC, d], f32, name=name, tag=name)

            if True:
                R2e = pp.tile([128, 2, BPC, 8], f32, tag="r2")
                R1e = pp.tile([128, 2, BPC, 8], f32, tag="r1")
                S1e = pp.tile([128, 2, BPC, 8], f32, tag="s1")
                R2ef = R2e.rearrange("p k b c -> p (k b c)")
                R1ef = R1e.rearrange("p k b c -> p (k b c)")
                S1ef = S1e.rearrange("p k b c -> p (k b c)")
                for m in range(2):
                    osl = slice(m * 128, (m + 1) * 128)
                    for kb in range(2):
                        isl = slice(kb * 128, (kb + 1) * 128)
                        st, sp = kb == 0, kb == 1
                        nc.tensor.matmul(R1ef[:, osl], Wb(m * 2 + kb),
                                         X1ef[:, isl], start=st, stop=sp)
                        nc.tensor.matmul(S1ef[:, osl], Wb(m * 2 + kb),
                                         X0ef[:, isl], start=st, stop=sp)
                        nc.tensor.matmul(R2ef[:, osl], Wb(10 + m * 2 + kb),
                                         X1ef[:, isl], start=st, stop=sp)

                # paired forward/mirrored diffs: half 0 = j=0 side (fwd),
                # half 1 = j=255 side (also forward-oriented: f7-f6 etc.)
                a1, b1, c1 = et("a1"), et("b1"), et("c1")
                a0, b0 = et("a0"), et("b0")
                nc.vector.tensor_sub(a1[:], E1[:, :, 1:8:6], E1[:, :, 0:7:6])
                nc.vector.tensor_sub(b1[:], E1[:, :, 2:7:4], E1[:, :, 1:6:4])
                nc.vector.tensor_sub(c1[:], E1[:, :, 3:6:2], E1[:, :, 2:5:2])
                nc.vector.tensor_sub(a0[:], E0[:, :, 1:8:6], E0[:, :, 0:7:6])
                nc.vector.tensor_sub(b0[:], E0[:, :, 2:7:4], E0[:, :, 1:6:4])

                # one-sided raw stencils (Z sign flips on the mirror half)
                q, Z = et("q"), et("Z")
                C1pe, C1ae = et("C1pe"), et("C1ae")
                stt(q[:], b1[:], 3.0, c1[:], MUL, SUB)      # 3b - c
                stt(Z[:], a1[:], -2.0, q[:], MUL, ADD)      # -2a + 3b - c
                stt(C1pe[:], a1[:], 3.0, b1[:], MUL, SUB)   # 3a - b
                stt(C1ae[:], a0[:], 3.0, b0[:], MUL, SUB)

                RP2 = R2e.rearrange("p k b c -> p (k b) c")
                RP1 = R1e.rearrange("p k b c -> p (k b) c")
                U4e, tme, t2e = et("U4e"), et("tme"), et("t2e")
                stt(U4e[:, :, 0:1], Z[:, :, 0:1], 4.0, RP2[:, :, 0:1], MUL, ADD)
                stt(U4e[:, :, 1:2], Z[:, :, 1:2], -4.0, RP2[:, :, 7:8], MUL, ADD)

                Scpe = epool.tile([128, 2, BPC, 8], f32)
                nc.scalar.copy(out=Scpe.rearrange("p k b c -> p (k b c)"),
                               in_=S1ef[:])
                SP = Scpe.rearrange("p k b c -> p (k b) c")

                stt(tme[:], E0[:, :, 0:8:7], 1.5, U4e[:], ADD, MUL)
                nc.vector.tensor_mul(t2e[:], SP[:, :, 0:8:7], RP1[:, :, 0:8:7])
                nc.vector.tensor_add(tme[:], tme[:], t2e[:])
                nc.vector.tensor_mul(C1ae[:], C1ae[:], C1pe[:])  # t3e in-place
                nc.vector.tensor_add(tme[:], tme[:], C1ae[:])
                rese = epool.tile([128, 2, BPC, 2], f32)
                nc.scalar.activation(
                    rese.rearrange("p k b e -> p (k b) e"), tme[:], COPY,
                    bias=-1.0, scale=-GAMMA)

            # ------------- main pipeline, 8 chunks of 2 images -------------
            if True:
                for c in range(CHUNKS):
                    b0c = c * BCH
                    if c == 0:
                        X0c, Xp = X0c0, Xp0
                    else:
                        X0c = pool.tile([128, 2, BCH, P], bf16, tag="x0",
                                        bufs=3)
                        Xp = pool.tile([128, FCH + 4], bf16, tag="x1",
                                       bufs=3)
                        nc.sync.dma_start(
                            out=Xp[:, 2:FCH + 2].rearrange(
                                "p (k b j) -> p k b j", k=2, b=BCH),
                            in_=xb[:, 1, :, b0c:b0c + BCH, :])
                        nc.sync.dma_start(out=X0c[:],
                                          in_=xb[:, 0, :, b0c:b0c + BCH, :])
                    X0f = X0c.rearrange("p k b j -> p (k b j)")
                    X1f = Xp[:, 2:FCH + 2]
                    C1p = pool.tile([128, FCH], bf16, tag="c1p", bufs=3)
                    C1a = pool.tile([128, FCH], bf16, tag="c1a", bufs=3)
                    t3b = pool.tile([128, FCH], bf16, tag="t3b", bufs=3)
                    t2b = pool.tile([128, FCH], bf16, tag="t2b", bufs=3)
                    rcp = pool.tile([128, FCH], bf16, tag="rcp", bufs=3)
                    tm = pool.tile([128, 2, BCH, P], bf16, tag="tm", bufs=3)
                    tmf = tm.rearrange("p k b j -> p (k b j)")

                    # column stencils, shifted layout (slot t = col t+1),
                    # all views 4-byte aligned -> 2x mode
                    nc.vector.tensor_sub(C1p[:, 0:FCH - 2], Xp[:, 4:FCH + 2],
                                         Xp[:, 2:FCH])
                    nc.vector.tensor_sub(C1a[:, 0:FCH - 2], X0f[:, 2:FCH],
                                         X0f[:, 0:FCH - 2])
                    nc.vector.tensor_mul(t3b[:], C1a[:], C1p[:])

                    res = pp.tile([128, 2, BCH, P], f32, name=f"res_{c}",
                                  tag="res", bufs=1)
                    resf = res.rearrange("p k b j -> p (k b j)")
                    for m in range(2):
                        R1s = pp.tile([128, 2 * P], f32, name=f"r1_{c}_{m}",
                                      tag="r1")
                        S1s = pp.tile([128, 2 * P], f32, name=f"s1_{c}_{m}",
                                      tag="s1")
                        U4s = pp.tile([128, 2 * P], f32, name=f"r2_{c}_{m}",
                                      tag="r2")
                        for kb in range(2):
                            st, sp = kb == 0, kb == 1
                            isl = slice(kb * (BCH * P), (kb + 1) * (BCH * P))
                            nc.tensor.matmul(R1s[:], Wb(m * 2 + kb),
                                             X1f[:, isl], start=st, stop=sp)
                            nc.tensor.matmul(S1s[:], Wb(m * 2 + kb),
                                             X0f[:, isl], start=st, stop=sp)
                            nc.tensor.matmul(U4s[:], Wb(4 + m * 2 + kb),
                                             X1f[:, isl], start=st, stop=False)
                        # column-neighbor sums via 4I with +-1-shifted rhs:
                        # U4 = W_R2@X1 + 4I@X1[+1] + 4I@X1[-1], all in PSUM
                        lo = m * (BCH * P)
                        hi = lo + 2 * P
                        nc.tensor.matmul(U4s[:], Wb(8),
                                         Xp[:, lo + 3:hi + 3],
                                         start=False, stop=False)
                        nc.tensor.matmul(U4s[:], Wb(8),
                                         Xp[:, lo + 1:hi + 1],
                                         start=False, stop=True)
                        # DVE can read only one PSUM operand per op: ScalarE
                        # evacuates R1 (shifted bf16), S1 stays in PSUM.
                        nc.scalar.copy(out=rcp[:, lo:hi - 1],
                                       in_=R1s[:, 1:2 * P])
                        # t2b slot t = S1*R1 at col t+1 (shifted bf16 out)
                        stt(t2b[:, lo:hi - 1], S1s[:, 1:2 * P], 1.0,
                            rcp[:, lo:hi - 1], MUL, MUL)
                        # tm = (X0 + 1.5) * U4  (PSUM source, bf16 out)
                        stt(tmf[:, lo:hi], X0f[:, lo:hi], 1.5, U4s[:],
                            ADD, MUL)

                    # gradient-product sum in shifted bf16 (2x), then the
                    # whole merge happens on TensorE in the res PSUM bank
                    # gradient-term sum on GpSimd (DVE is the hotter engine)
                    nc.gpsimd.tensor_add(t2b[:], t2b[:], t3b[:])
                    # PSUM-bank-sized (N<=512) merge matmuls
                    H = FCH // 2
                    nc.tensor.matmul(resf[:, 1:H], Wb(9), tmf[:, 1:H],
                                     start=True, stop=False)
                    nc.tensor.matmul(resf[:, 1:H], Wb(9), t2b[:, 0:H - 1],
                                     start=False, stop=True)
                    nc.tensor.matmul(resf[:, H:FCH], Wb(9), tmf[:, H:FCH],
                                     start=True, stop=False)
                    nc.tensor.matmul(resf[:, H:FCH], Wb(9),
                                     t2b[:, H - 1:FCH - 1],
                                     start=False, stop=True)

                    outt = pool.tile([128, 2, BCH, P], bf16, tag="out", bufs=3)
                    nc.scalar.activation(outt[:, :, :, 1:P - 1],
                                         res[:, :, :, 1:P - 1], COPY,
                                         bias=-1.0, scale=-GAMMA)
                    # edge columns j=0,255 from the edge pipeline (one copy)
                    nc.scalar.copy(out=outt[:, :, :, 0:P:P - 1],
                                   in_=rese[:, :, b0c:b0c + BCH, :])
                    nc.gpsimd.dma_start(
                        out=yout[:, :, b0c:b0c + BCH, :], in_=outt[:])

    nc.compile()
    return nc


def _get_program():
    if "nc" not in _cache:
        _cache["nc"] = _build_program()
        _cache["wtbd"] = _weights_bf16()
    return _cache["nc"], _cache["wtbd"]


def _shard_inputs(x0_pred):
    import ml_dtypes
    x = np.ascontiguousarray(np.asarray(x0_pred, dtype=np.float32))
    _, wtbd = _get_program()
    in_maps = []
    for i in range(NCORES):
        shard = x[i * BPC:(i + 1) * BPC]                      # [16,2,256,256]
        arr = shard.reshape(BPC, 2, 2, 128, P).transpose(3, 1, 2, 0, 4)
        xbi = np.ascontiguousarray(arr).astype(ml_dtypes.bfloat16)
        cols = [0, 1, 2, 3, P - 4, P - 3, P - 2, P - 1]
        xe = np.ascontiguousarray(xbi[:, :, :, :, cols])
        in_maps.append({"xe": xe, "xb": xbi, "wtbd": wtbd})
    return in_maps


def _unshard(results):
    outs = []
    for i in range(NCORES):
        y = np.asarray(results[i]["yout"], dtype=np.float32)  # [128,2,16,256]
        outs.append(y.transpose(2, 1, 0, 3).reshape(BPC, 1, P, P))
    return np.ascontiguousarray(np.concatenate(outs, axis=0))


def _run(x0_pred, trace=False, tmpdir=None):
    import time
    from concourse.bass_utils import run_bass_kernel_spmd
    nc = _get_program()[0]
    in_maps = _shard_inputs(x0_pred)
    try:
        res = run_bass_kernel_spmd(nc, in_maps, list(range(NCORES)),
                                   trace=trace, tmpdir=tmpdir)
    except Exception:
        # transient NRT execution failures have been observed; one retry
        time.sleep(2.0)
        res = run_bass_kernel_spmd(nc, in_maps, list(range(NCORES)),
                                   trace=trace, tmpdir=tmpdir)
    return _unshard(res.results), res


def kernel(x0_pred):
    out, _ = _run(x0_pred, trace=False)
    return out
